# revision 3
# baseline (speedup 1.0000x reference)
"""Trainium2 Bass kernel for nn_CropCrossEntropy.

Reference computation (see reference.py):
    gt[i, y, x] = 1 inside the inclusive box [y0:y1, x0:x1] of image i, else 0
    loss = -(log(mp)*gt + log1p(-mp)*(1-gt)).mean()

Reformulation used here: with q = mp inside the box and q = 1-mp outside,
    loss = -mean(ln q),   q = sigma*(mp - 0.5) + 0.5,   sigma = 2*gt - 1.

sigma is rank-3 per image-tile (row-indicator x col-indicator outer products,
values in {-1,+1} exactly representable in bf16), so the TensorEngine builds
it in PSUM from tiny host-precomputed bf16 factors. Per element the device
then does ONE VectorE op  u = (mp - 0.5) * sigma  (scalar_tensor_tensor) and
ONE ScalarE op  ln(u + 0.5)  (activation with free affine bias + fused
per-partition accumulation). The kernel is HBM-bandwidth bound (~16.8 MB/core).

Sharding: data-parallel over the fused (b*r)=512 image dim, 64 images/core on
8 cores; each core returns per-partition partial sums of ln q; the host does
the final (tiny) reduction and the mean.

Per-core layout: image i (256x256 f32) is viewed as a [128, 512] tile whose
partition p holds image rows 2p and 2p+1; images are processed in chunks of 4
([128, 2048], 4 PSUM banks, double-buffered = all 8 banks).
"""

from contextlib import ExitStack

import ml_dtypes
import numpy as np

import concourse.bass as bass
import concourse.tile as tile
from concourse import bacc, mybir
from concourse.bass_utils import run_bass_kernel_spmd

N_CORES = 8
B, R, H, W = 32, 16, 256, 256
IMGS = B * R                      # 512
IMGS_PER_CORE = IMGS // N_CORES   # 64
P = 128
FREE = H * W // P                 # 512 (2 image rows per partition)
CHUNK_IMGS = 4
N_CHUNKS = IMGS_PER_CORE // CHUNK_IMGS  # 16
CHUNK_FREE = CHUNK_IMGS * FREE    # 2048
N_ELEMS = IMGS * H * W

_cached_nc = None


def _build_nc():
    """Build + compile the (single-program SPMD) Bass kernel."""
    nc = bacc.Bacc("TRN2", target_bir_lowering=False, debug=False)

    mp = nc.dram_tensor(
        "mp", [IMGS_PER_CORE * P, FREE], mybir.dt.float32, kind="ExternalInput"
    ).ap()
    mlhs = nc.dram_tensor(
        "mlhs", [3, IMGS_PER_CORE * P], mybir.dt.bfloat16, kind="ExternalInput"
    ).ap()
    mrhs = nc.dram_tensor(
        "mrhs", [3, IMGS_PER_CORE * FREE], mybir.dt.bfloat16, kind="ExternalInput"
    ).ap()
    acc_out = nc.dram_tensor(
        "acc", [P, N_CHUNKS], mybir.dt.float32, kind="ExternalOutput"
    ).ap()

    with tile.TileContext(nc) as tc, ExitStack() as ctx:
        mask_pool = ctx.enter_context(tc.tile_pool(name="masks", bufs=1))
        mp_pool = ctx.enter_context(tc.tile_pool(name="mp", bufs=3))
        u_pool = ctx.enter_context(tc.tile_pool(name="u", bufs=3))
        scr_pool = ctx.enter_context(tc.tile_pool(name="scr", bufs=2))
        acc_pool = ctx.enter_context(tc.tile_pool(name="acc", bufs=1))
        ps_pool = ctx.enter_context(tc.tile_pool(name="sig", bufs=2, space="PSUM"))

        mlhs_t = mask_pool.tile([3, IMGS_PER_CORE * P], mybir.dt.bfloat16)
        mrhs_t = mask_pool.tile([3, IMGS_PER_CORE * FREE], mybir.dt.bfloat16)
        nc.sync.dma_start(mlhs_t[:], mlhs[:])
        nc.sync.dma_start(mrhs_t[:], mrhs[:])

        half_t = mask_pool.tile([P, 1], mybir.dt.float32)
        nc.gpsimd.memset(half_t[:], 0.5)

        acc_t = acc_pool.tile([P, N_CHUNKS], mybir.dt.float32)

        for c in range(N_CHUNKS):
            mp_t = mp_pool.tile([P, CHUNK_FREE], mybir.dt.float32)
            for i in range(CHUNK_IMGS):
                img = c * CHUNK_IMGS + i
                nc.sync.dma_start(
                    mp_t[:, i * FREE : (i + 1) * FREE],
                    mp[img * P : (img + 1) * P, :],
                )

            # sigma = 2*gt - 1 in PSUM via K=3 outer products per image
            sg_t = ps_pool.tile([P, CHUNK_FREE], mybir.dt.float32)
            for i in range(CHUNK_IMGS):
                img = c * CHUNK_IMGS + i
                nc.tensor.matmul(
                    sg_t[:, i * FREE : (i + 1) * FREE],
                    mlhs_t[:, img * P : (img + 1) * P],
                    mrhs_t[:, img * FREE : (img + 1) * FREE],
                    start=True,
                    stop=True,
                )

            # u = (mp - 0.5) * sigma  (one DVE instruction)
            u_t = u_pool.tile([P, CHUNK_FREE], mybir.dt.float32)
            nc.vector.scalar_tensor_tensor(
                u_t[:],
                mp_t[:],
                0.5,
                sg_t[:],
                mybir.AluOpType.subtract,
                mybir.AluOpType.mult,
            )

            # ln(u + 0.5) with fused per-partition sum into acc column c
            scr_t = scr_pool.tile([P, CHUNK_FREE], mybir.dt.float32)
            nc.scalar.activation(
                scr_t[:],
                u_t[:],
                mybir.ActivationFunctionType.Ln,
                bias=half_t[:, 0:1],
                scale=1.0,
                accum_out=acc_t[:, c : c + 1],
            )

        nc.sync.dma_start(acc_out[:], acc_t[:])

    nc.compile()
    return nc


def _get_nc():
    global _cached_nc
    if _cached_nc is None:
        _cached_nc = _build_nc()
    return _cached_nc


def _make_in_maps(mask_pred, pos_gt):
    mp = np.ascontiguousarray(np.asarray(mask_pred), dtype=np.float32).reshape(
        IMGS, H * W
    )
    pg = np.asarray(pos_gt).reshape(IMGS, 4).astype(np.int64)
    rows = np.arange(H)[None, :]
    cols = np.arange(W)[None, :]
    y0, x0, y1, x1 = (pg[:, k][:, None] for k in range(4))
    rowind = ((rows >= y0) & (rows <= y1)).astype(np.float32)  # (512, 256)
    colind = ((cols >= x0) & (cols <= x1)).astype(np.float32)  # (512, 256)

    in_maps = []
    for cid in range(N_CORES):
        sl = slice(cid * IMGS_PER_CORE, (cid + 1) * IMGS_PER_CORE)
        mp_c = mp[sl].reshape(IMGS_PER_CORE * P, FREE)
        ri = rowind[sl]
        ci = colind[sl]
        # lhsT rows: [rowind(even rows), rowind(odd rows), ones]
        lhs = np.empty((IMGS_PER_CORE, 3, P), np.float32)
        lhs[:, 0, :] = ri[:, 0::2]
        lhs[:, 1, :] = ri[:, 1::2]
        lhs[:, 2, :] = 1.0
        # rhs rows: [2*colind | 0, 0 | 2*colind, -1]  ->  sigma = 2*gt - 1
        rhs = np.zeros((IMGS_PER_CORE, 3, FREE), np.float32)
        rhs[:, 0, 0:W] = 2.0 * ci
        rhs[:, 1, W : 2 * W] = 2.0 * ci
        rhs[:, 2, :] = -1.0
        mlhs = np.ascontiguousarray(lhs.transpose(1, 0, 2)).reshape(3, -1)
        mrhs = np.ascontiguousarray(rhs.transpose(1, 0, 2)).reshape(3, -1)
        in_maps.append(
            {
                "mp": mp_c,
                "mlhs": mlhs.astype(ml_dtypes.bfloat16),
                "mrhs": mrhs.astype(ml_dtypes.bfloat16),
            }
        )
    return in_maps


def _run(mask_pred, pos_gt, trace=False, **run_kwargs):
    nc = _get_nc()
    in_maps = _make_in_maps(mask_pred, pos_gt)
    res = run_bass_kernel_spmd(
        nc, in_maps, core_ids=list(range(N_CORES)), trace=trace, **run_kwargs
    )
    total = 0.0
    for r in res.results:
        total += float(np.sum(np.asarray(r["acc"], dtype=np.float64)))
    loss = np.float32(-(total / N_ELEMS))
    return loss, res


def kernel(mask_pred, pos_gt):
    loss, _ = _run(mask_pred, pos_gt, trace=False)
    return loss


# revision 4
# speedup vs baseline: 1.2108x; 1.2108x over previous
"""Trainium2 Bass kernel for nn_CropCrossEntropy.

Reference computation (see reference.py):
    gt[i, y, x] = 1 inside the inclusive box [y0:y1, x0:x1] of image i, else 0
    loss = -(log(mp)*gt + log1p(-mp)*(1-gt)).mean()

Reformulation used here: with q = mp inside the box and q = 1-mp outside,
    loss = -mean(ln q),   q = sigma*(mp - 0.5) + 0.5,   sigma = 2*gt - 1.

sigma is a small-rank product of row/col box indicators, exactly
representable in bf16, so the TensorEngine builds it in PSUM from tiny
host-precomputed factors. Per element the device then does ONE VectorE op
u = (mp - 0.5) * sigma (scalar_tensor_tensor) and ONE ScalarE op
ln(2u + 1) = ln 2 + ln q (activation, free affine scale/bias, fused
per-partition accumulation); the host subtracts N*ln2. The kernel is
HBM-bandwidth bound (~16.8 MB/core).

Sharding: data-parallel over the fused (b*r)=512 image dim, 64 images/core
on 8 cores; each core returns per-partition partial sums; the host does the
final (tiny) reduction and the mean.

Per-core layout ("flat"): the 4 images of a chunk are one contiguous 1 MB
DRAM block viewed as [128, 2048] — partition p holds 2048 consecutive
floats = 8 consecutive rows of image (p//32). 8 KB contiguous DMA lines
per partition maximize DMA engine packet efficiency. For PSUM bank b
(columns [512b, 512b+512)), element (p, j') is image i=p//32, row
8*(p%32) + 2b + (j'//256), col j'%256 — so sigma for a bank is a K=9
matmul: rows (2i+h) pair [p//32==i]*rowind_i(8*(p%32)+2b+h) on the lhsT
side with 2*colind_i in column-half h on the rhs side, plus a constant
(ones x -1) row.
"""

from contextlib import ExitStack

import ml_dtypes
import numpy as np

import concourse.bass as bass
import concourse.tile as tile
from concourse import bacc, mybir
from concourse.bass_utils import run_bass_kernel_spmd

N_CORES = 8
B, R, H, W = 32, 16, 256, 256
IMGS = B * R                      # 512
IMGS_PER_CORE = IMGS // N_CORES   # 64
P = 128
CHUNK_IMGS = 4
N_CHUNKS = IMGS_PER_CORE // CHUNK_IMGS  # 16
CHUNK_FREE = CHUNK_IMGS * H * W // P    # 2048 (8 image rows per partition)
BANK = 512
N_BANKS = CHUNK_FREE // BANK      # 4
K = 9                             # mask rank: 4 images x 2 col-halves + const
N_ELEMS = IMGS * H * W
LN2 = float(np.log(2.0))

_cached_nc = None


def _build_nc():
    """Build + compile the (single-program SPMD) Bass kernel."""
    nc = bacc.Bacc("TRN2", target_bir_lowering=False, debug=False)

    mp = nc.dram_tensor(
        "mp", [N_CHUNKS * P, CHUNK_FREE], mybir.dt.float32, kind="ExternalInput"
    ).ap()
    mlhs = nc.dram_tensor(
        "mlhs", [K, N_CHUNKS * N_BANKS * P], mybir.dt.bfloat16, kind="ExternalInput"
    ).ap()
    mrhs = nc.dram_tensor(
        "mrhs", [K, N_CHUNKS * BANK], mybir.dt.bfloat16, kind="ExternalInput"
    ).ap()
    acc_out = nc.dram_tensor(
        "acc", [P, N_CHUNKS], mybir.dt.float32, kind="ExternalOutput"
    ).ap()

    with tile.TileContext(nc) as tc, ExitStack() as ctx:
        mask_pool = ctx.enter_context(tc.tile_pool(name="masks", bufs=1))
        mp_pool = ctx.enter_context(tc.tile_pool(name="mp", bufs=3))
        u_pool = ctx.enter_context(tc.tile_pool(name="u", bufs=3))
        scr_pool = ctx.enter_context(tc.tile_pool(name="scr", bufs=2))
        acc_pool = ctx.enter_context(tc.tile_pool(name="acc", bufs=1))
        ps_pool = ctx.enter_context(tc.tile_pool(name="sig", bufs=2, space="PSUM"))

        mlhs_t = mask_pool.tile([K, N_CHUNKS * N_BANKS * P], mybir.dt.bfloat16)
        mrhs_t = mask_pool.tile([K, N_CHUNKS * BANK], mybir.dt.bfloat16)
        nc.sync.dma_start(mlhs_t[:], mlhs[:])
        nc.sync.dma_start(mrhs_t[:], mrhs[:])

        acc_t = acc_pool.tile([P, N_CHUNKS], mybir.dt.float32)

        for c in range(N_CHUNKS):
            mp_t = mp_pool.tile([P, CHUNK_FREE], mybir.dt.float32)
            nc.sync.dma_start(mp_t[:], mp[c * P : (c + 1) * P, :])

            # sigma = 2*gt - 1 in PSUM, one K=9 matmul per bank
            sg_t = ps_pool.tile([P, CHUNK_FREE], mybir.dt.float32)
            for b in range(N_BANKS):
                nc.tensor.matmul(
                    sg_t[:, b * BANK : (b + 1) * BANK],
                    mlhs_t[:, (c * N_BANKS + b) * P : (c * N_BANKS + b + 1) * P],
                    mrhs_t[:, c * BANK : (c + 1) * BANK],
                    start=True,
                    stop=True,
                )

            # u = (mp - 0.5) * sigma  (one DVE instruction)
            u_t = u_pool.tile([P, CHUNK_FREE], mybir.dt.float32)
            nc.vector.scalar_tensor_tensor(
                u_t[:],
                mp_t[:],
                0.5,
                sg_t[:],
                mybir.AluOpType.subtract,
                mybir.AluOpType.mult,
            )

            # ln(2u + 1) = ln2 + ln(q), fused per-partition sum into acc col c
            scr_t = scr_pool.tile([P, CHUNK_FREE], mybir.dt.float32)
            nc.scalar.activation(
                scr_t[:],
                u_t[:],
                mybir.ActivationFunctionType.Ln,
                bias=1.0,
                scale=2.0,
                accum_out=acc_t[:, c : c + 1],
            )

        nc.sync.dma_start(acc_out[:], acc_t[:])

    nc.compile()
    return nc


def _get_nc():
    global _cached_nc
    if _cached_nc is None:
        _cached_nc = _build_nc()
    return _cached_nc


def _make_in_maps(mask_pred, pos_gt):
    mp = np.ascontiguousarray(np.asarray(mask_pred), dtype=np.float32).reshape(
        IMGS, H * W
    )
    pg = np.asarray(pos_gt).reshape(IMGS, 4).astype(np.int64)
    rows = np.arange(H)[None, :]
    cols = np.arange(W)[None, :]
    y0, x0, y1, x1 = (pg[:, k][:, None] for k in range(4))
    rowind = ((rows >= y0) & (rows <= y1)).astype(np.float32)  # (512, 256)
    colind = ((cols >= x0) & (cols <= x1)).astype(np.float32)  # (512, 256)

    # lhsT row for bank b: 8*(p%32) + 2b + h, p in [32i, 32i+32)
    q32 = np.arange(32)
    bank_rows = 8 * q32[None, :] + 2 * np.arange(N_BANKS)[:, None]  # (4, 32)

    in_maps = []
    for cid in range(N_CORES):
        sl = slice(cid * IMGS_PER_CORE, (cid + 1) * IMGS_PER_CORE)
        mp_c = mp[sl].reshape(N_CHUNKS * P, CHUNK_FREE)
        rc = rowind[sl].reshape(N_CHUNKS, CHUNK_IMGS, H)
        cc = colind[sl].reshape(N_CHUNKS, CHUNK_IMGS, W)

        lhs = np.zeros((N_CHUNKS, N_BANKS, K, P), np.float32)
        rhs = np.zeros((N_CHUNKS, K, BANK), np.float32)
        for i in range(CHUNK_IMGS):
            for h in range(2):
                # (chunks, banks, 32)
                lhs[:, :, 2 * i + h, 32 * i : 32 * (i + 1)] = rc[:, i][
                    :, bank_rows + h
                ]
                rhs[:, 2 * i + h, 256 * h : 256 * (h + 1)] = 2.0 * cc[:, i]
        lhs[:, :, 8, :] = 1.0
        rhs[:, 8, :] = -1.0

        mlhs = np.ascontiguousarray(
            lhs.reshape(N_CHUNKS * N_BANKS, K, P).transpose(1, 0, 2)
        ).reshape(K, -1)
        mrhs = np.ascontiguousarray(rhs.transpose(1, 0, 2)).reshape(K, -1)
        in_maps.append(
            {
                "mp": mp_c,
                "mlhs": mlhs.astype(ml_dtypes.bfloat16),
                "mrhs": mrhs.astype(ml_dtypes.bfloat16),
            }
        )
    return in_maps


def _run(mask_pred, pos_gt, trace=False, **run_kwargs):
    nc = _get_nc()
    in_maps = _make_in_maps(mask_pred, pos_gt)
    res = run_bass_kernel_spmd(
        nc, in_maps, core_ids=list(range(N_CORES)), trace=trace, **run_kwargs
    )
    total = 0.0
    for r in res.results:
        total += float(np.sum(np.asarray(r["acc"], dtype=np.float64)))
    # acc sums ln(2u+1) = ln2 + ln(q): subtract the known N*ln2 shift
    loss = np.float32(-((total - N_ELEMS * LN2) / N_ELEMS))
    return loss, res


def kernel(mask_pred, pos_gt):
    loss, _ = _run(mask_pred, pos_gt, trace=False)
    return loss
